# revision 8
# baseline (speedup 1.0000x reference)
"""Trainium2 Bass kernel for ContinuousConvEmbedding (Open3D-style).

Reformulation: out[f,i] = relu(bias + sum_{m,j} Phi[j,m,f] * mono_m[j,i]),
with Phi = features @ K2 precomputed on host (K2 = M^x3-transformed kernel),
mono_m = w * prod_a basis_{m_a}(d_a), basis in {1, d, s=|d|},
d = rel * r/linf (ball->cube radial map), w = relu(1-r2)^3.
r2 is computed exactly via a K=15 hi/lo-split bf16 PE matmul; the 432
channel matmuls (27 bins x 16 j-tiles, N=256) accumulate out in PSUM.

Engine split tuned to MEASURED TRN2 per-op costs (v3):
  DVE  (0.52 ns/el TT-2x, 0.26 ns/el TS-4x, ~350 ns/op fixed): rel/linf/
       scale chain, most channel products, abs via 4x bitand.
  Act  (~0.92 ns/el + ~0.5 us/op): sqrt/relu/square/copy + late abs
       channels; emitted r2-dependent ops first so Act is never
       head-blocked on the reciprocal chain.
  Pool (gpsimd, ~2.1 ns/el — 2.6x worse than the cost model!): only two
       late leaf channel groups (ds02, L*d2) that no other engine reads.
  PE:  448 matmuls/iter; channel matmuls deferred one block so PE streams
       block q-1 while DVE/Act/Pool run block q's geometry.

Geometry is blocked over j (4 blocks x 4 j-tiles, free dim 1024); all
cross-engine tiles double-buffered for cross-block overlap.

Sharding: output points i across 8 cores (256 each); inputs replicated.
No collectives.
"""
import sys

sys.path.insert(0, "/opt/trn_rl_repo")

import numpy as np
import ml_dtypes

import concourse.bass as bass
import concourse.mybir as mybir
import concourse.tile as tile
from concourse import bacc
from concourse.bass_utils import run_bass_kernel_spmd

F32 = mybir.dt.float32
BF16 = mybir.dt.bfloat16
U16 = mybir.dt.uint16
AF = mybir.ActivationFunctionType
ALU = mybir.AluOpType

N_CORES = 8
N_IN = 2048
N_OUT = 2048
C_IN = 8
C_OUT = 64
K3 = 27

NI = N_OUT // N_CORES          # 256 output points per core
NJT = N_IN // 128              # 16 j-tiles
JT_PER_Q = 4                   # j-tiles per geometry block
NQ = NJT // JT_PER_Q           # 4 blocks
FQ = JT_PER_Q * NI             # 1024 free elems per geometry op

SQRT_BIAS = 2e-4               # guards rsqrt against r2 rounding below zero
LPE_FLOOR = 1e-6               # guards 1/linf against coincident points

M_BASIS = np.array([[0.0, -0.5, 0.5],
                    [1.0, 0.0, -1.0],
                    [0.0, 0.5, 0.5]], np.float32)

# channel -> kernel bin m = b0*9 + b1*3 + b2  (b: 0=1, 1=d, 2=s per axis)
M_W = 0
M_A = (9, 18, 3, 6, 1, 2)             # A_d0,A_s0,A_d1,A_s1,A_d2,A_s2
M_P = (12, 21, 15, 24, 10, 19, 4, 7)  # dd01,sd01,ds01,ss01,dd02,sd02,dd12,sd12
M_Q = (11, 5, 20, 8)                  # ds02,ds12,ss02,ss12
# L slots follow P-slot order: P[0:4]*d2 -> ddd,sdd,dsd,ssd ; P[0:3]*s2 ->
# dds,sds,dss ; |ddd| -> sss
M_L = (13, 22, 16, 25, 14, 23, 17, 26)


def build_nc(repeat: int = 1, variant: str = "full"):
    nc = bacc.Bacc("TRN2", target_bir_lowering=False, debug=False,
                   num_devices=N_CORES)
    pin_d = nc.dram_tensor("pin_sc", [128, NJT * 3], F32,
                           kind="ExternalInput").ap()
    nbcast_d = nc.dram_tensor("nbcast", [128, 3 * NI], BF16,
                              kind="ExternalInput").ap()
    phi_d = nc.dram_tensor("phi", [N_IN, K3 * C_OUT], BF16,
                           kind="ExternalInput").ap()
    lhsT15_d = nc.dram_tensor("lhsT15", [15, N_IN], BF16,
                              kind="ExternalInput").ap()
    rhs15_d = nc.dram_tensor("rhs15", [15, NI], BF16,
                             kind="ExternalInput").ap()
    bias_d = nc.dram_tensor("bias", [C_OUT, 1], F32, kind="ExternalInput").ap()
    y_d = nc.dram_tensor("y", [C_OUT, NI], F32, kind="ExternalOutput").ap()

    do_mm = variant in ("full", "mm")
    do_geo = variant in ("full", "geo")

    with tile.TileContext(nc) as tc:
        with tc.tile_pool(name="const", bufs=1) as constp, \
             tc.tile_pool(name="early", bufs=1) as early, \
             tc.tile_pool(name="geo", bufs=2) as geo, \
             tc.tile_pool(name="phip", bufs=2) as phip, \
             tc.tile_pool(name="outp", bufs=2) as outp, \
             tc.tile_pool(name="psr2", bufs=2, space="PSUM") as psr2, \
             tc.tile_pool(name="psout", bufs=2, space="PSUM") as psout:

            # ---- resident constants ----
            pin = constp.tile([128, NJT, 3], F32)
            nbcast = constp.tile([128, 3, 1, NI], BF16)
            lhsT15 = constp.tile([15, N_IN], BF16)
            rhs15 = constp.tile([15, NI], BF16)
            bias = constp.tile([C_OUT, 1], F32)
            sqb = constp.tile([128, 1], F32)
            nc.sync.dma_start(pin[:], pin_d[:])
            nc.sync.dma_start(nbcast[:], nbcast_d[:])
            nc.sync.dma_start(lhsT15[:], lhsT15_d[:])
            nc.sync.dma_start(rhs15[:], rhs15_d[:])
            nc.sync.dma_start(bias[:], bias_d[:])
            nc.gpsimd.memset(sqb[:], SQRT_BIAS)
            phi_ap = phi_d.rearrange("(a p) x -> p a x", p=128)
            if not do_geo:
                fkm = constp.tile([128, JT_PER_Q, NI], BF16)
                nc.gpsimd.memset(fkm[:], 0.01)

            def body(_iv=None):
                out_acc = psout.tile([C_OUT, NI], F32, tag="oacc",
                                     name="out_acc")
                n_mm = [0]
                tot_mm = NQ * K3 * JT_PER_Q

                def mm_for(m, rhs_ap, phiq):
                    """4 matmuls (one per j-tile of the block) for bin m.
                    rhs_ap: [128, JT_PER_Q, NI]."""
                    if not do_mm:
                        n_mm[0] += JT_PER_Q
                        return
                    for jl in range(JT_PER_Q):
                        nc.tensor.matmul(
                            out_acc[:],
                            phiq[:, jl, m * C_OUT:(m + 1) * C_OUT],
                            rhs_ap[:, jl, :],
                            start=(n_mm[0] == 0),
                            stop=(n_mm[0] == tot_mm - 1))
                        n_mm[0] += 1

                def flush(mm_q, phiq):
                    for m, ap in mm_q:
                        mm_for(m, ap, phiq)

                deferred = [None] * NQ   # per-block (mm_q list, phiq)

                for q in range(NQ):
                    # streamed Phi slab for this block's j-tiles
                    phiq = phip.tile([128, JT_PER_Q, K3 * C_OUT], BF16,
                                     tag="phiq", name="phiq")
                    if do_mm:
                        for jl in range(JT_PER_Q):
                            nc.sync.dma_start(
                                phiq[:, jl:jl + 1, :],
                                phi_ap[:, q * JT_PER_Q + jl:
                                       q * JT_PER_Q + jl + 1, :])

                    # ---- r2 via K=15 hi/lo stacked bf16 matmul (PE) ----
                    if do_geo:
                        r2q = psr2.tile([128, JT_PER_Q, NI], F32, tag="r2q",
                                        name="r2q")
                        for jl in range(JT_PER_Q):
                            jt = q * JT_PER_Q + jl
                            nc.tensor.matmul(r2q[:, jl, :],
                                             lhsT15[:, jt * 128:(jt + 1) * 128],
                                             rhs15[:], start=True, stop=True)

                    # flush previous block's deferred channel matmuls: they
                    # stream on PE while this block's geometry runs on
                    # DVE/Act/Pool.
                    if q > 0 and deferred[q - 1] is not None:
                        pmm, pphiq = deferred[q - 1]
                        flush(pmm, pphiq)
                        deferred[q - 1] = None
                    if not do_geo and do_mm:
                        flush([(m, fkm[:]) for m in range(K3)], phiq)

                    if not do_geo:
                        continue

                    # ---- geometry block q ----
                    # rel_a[j, (jl,i)] = pin_a[j] - pout_a[i]   (1 fused TT)
                    rel3 = geo.tile([128, 3, JT_PER_Q, NI], BF16, tag="rel3",
                                    name="rel3")
                    for a in range(3):
                        for jl in range(JT_PER_Q):
                            jt = q * JT_PER_Q + jl
                            nc.vector.tensor_scalar_add(
                                rel3[:, a, jl], nbcast[:, a, 0],
                                pin[:, jt, a:a + 1])

                    # linf = max(|rel0|,|rel1|,|rel2|+floor)  (f32 for the
                    # fp32-only reciprocal op; abs via 4x bitand)
                    ab3 = early.tile([128, 3, JT_PER_Q, NI], BF16, tag="ab3",
                                     name="ab3")
                    nc.vector.tensor_scalar(
                        ab3.bitcast(U16)[:], rel3.bitcast(U16)[:],
                        0x7FFF, None, ALU.bitwise_and)
                    # Act (sqrt table): r = sqrt(r2+eps), u, u2 — emitted
                    # first so Act isn't head-blocked on the lpe chain
                    r_ = early.tile([128, JT_PER_Q, NI], BF16, tag="r_",
                                    name="r_")
                    nc.scalar.activation(r_[:], r2q[:], AF.Sqrt,
                                         bias=sqb[:, 0:1])
                    u = early.tile([128, JT_PER_Q, NI], BF16, tag="u", name="u")
                    nc.scalar.activation(u[:], r2q[:], AF.Relu,
                                         bias=1.0, scale=-1.0)
                    u2 = early.tile([128, JT_PER_Q, NI], BF16, tag="u2",
                                    name="u2")
                    nc.scalar.activation(u2[:], u[:], AF.Square)

                    linf1 = early.tile([128, JT_PER_Q, NI], BF16, tag="linf1",
                                       name="linf1")
                    nc.vector.tensor_tensor(linf1[:], ab3[:, 0], ab3[:, 1],
                                            ALU.max)
                    lpe32 = early.tile([128, JT_PER_Q, NI], F32, tag="lpe32",
                                       name="lpe32")
                    nc.vector.scalar_tensor_tensor(
                        lpe32[:], ab3[:, 2], LPE_FLOOR, linf1[:],
                        ALU.add, ALU.max)
                    rl32 = lpe32
                    nc.vector.reciprocal_approx_fast(rl32[:], lpe32[:])
                    rlb = early.tile([128, JT_PER_Q, NI], BF16, tag="rlb",
                                     name="rlb")
                    nc.scalar.activation(rlb[:], rl32[:], AF.Copy)

                    # w = u2*u  (stream channel m=0)
                    wch = geo.tile([128, 1, JT_PER_Q, NI], BF16, tag="wch",
                                   name="wch")
                    nc.vector.tensor_tensor(wch[:, 0], u2[:], u[:], ALU.mult)

                    # scale = r/linf
                    scale = early.tile([128, 1, JT_PER_Q, NI], BF16,
                                       tag="scale", name="scale")
                    nc.vector.tensor_tensor(scale[:, 0], r_[:], rlb[:],
                                            ALU.mult)

                    # d_a = rel_a * scale ; s1,s2 = |d1|,|d2|
                    dblk = geo.tile([128, 3, JT_PER_Q, NI], BF16, tag="dblk",
                                    name="dblk")
                    nc.vector.tensor_tensor(
                        dblk[:], rel3[:],
                        scale[:].to_broadcast([128, 3, JT_PER_Q, NI]),
                        ALU.mult)
                    sblk = geo.tile([128, 2, JT_PER_Q, NI], BF16, tag="sblk",
                                    name="sblk")
                    nc.scalar.activation(sblk[:], dblk[:, 1:3], AF.Abs)

                    # A: interleaved (A_d0, A_s0, A_d1, A_s1, A_d2, A_s2)
                    A = geo.tile([128, 6, JT_PER_Q, NI], BF16, tag="A",
                                 name="A")
                    nc.vector.tensor_tensor(
                        A[:, 0:6:2], dblk[:],
                        wch[:].to_broadcast([128, 3, JT_PER_Q, NI]), ALU.mult)
                    nc.scalar.activation(A[:, 1:6:2], A[:, 0:6:2], AF.Abs)

                    mm_q = [(M_W, wch[:, 0])]
                    for k in range(6):
                        mm_q.append((M_A[k], A[:, k]))

                    # P: dd01,sd01 | ds01 | ss01 | dd02,sd02,dd12,sd12
                    P = geo.tile([128, 8, JT_PER_Q, NI], BF16, tag="P",
                                 name="P")
                    d1b = dblk[:, 1:2]
                    d2b = dblk[:, 2:3]
                    s2b = sblk[:, 1:2]
                    nc.vector.tensor_tensor(
                        P[:, 0:2], A[:, 0:2],
                        d1b.to_broadcast([128, 2, JT_PER_Q, NI]), ALU.mult)
                    nc.vector.tensor_tensor(P[:, 2], A[:, 0], sblk[:, 0],
                                            ALU.mult)
                    nc.vector.tensor_tensor(
                        P[:, 4:8], A[:, 0:4],
                        d2b.to_broadcast([128, 4, JT_PER_Q, NI]), ALU.mult)
                    nc.scalar.activation(P[:, 3], P[:, 0], AF.Abs)

                    # Q: ds02,ds12 (DVE) | ss02,ss12 (Act abs)
                    Qt = geo.tile([128, 4, JT_PER_Q, NI], BF16, tag="Qt",
                                  name="Qt")
                    nc.gpsimd.tensor_tensor(Qt[:, 0], A[:, 0], sblk[:, 1],
                                            ALU.mult)
                    nc.vector.tensor_tensor(Qt[:, 1], A[:, 2], sblk[:, 1],
                                            ALU.mult)
                    nc.scalar.activation(Qt[:, 2:4], P[:, 4:8:2], AF.Abs)

                    # L: ddd,sdd,dsd,ssd (Pool — only Pool work in the block)
                    # | dds,sds,dss (DVE) | sss
                    L = geo.tile([128, 8, JT_PER_Q, NI], BF16, tag="L",
                                 name="L")
                    nc.gpsimd.tensor_tensor(
                        L[:, 0:4], P[:, 0:4],
                        d2b.to_broadcast([128, 4, JT_PER_Q, NI]), ALU.mult)
                    nc.vector.tensor_tensor(
                        L[:, 4:7], P[:, 0:3],
                        s2b.to_broadcast([128, 3, JT_PER_Q, NI]), ALU.mult)
                    nc.scalar.activation(L[:, 7], L[:, 0], AF.Abs)

                    for k in range(8):
                        mm_q.append((M_P[k], P[:, k]))
                    for k in range(4):
                        mm_q.append((M_Q[k], Qt[:, k]))
                    for k in range(8):
                        mm_q.append((M_L[k], L[:, k]))
                    deferred[q] = (mm_q, phiq)

                if do_geo and deferred[NQ - 1] is not None:
                    pmm, pphiq = deferred[NQ - 1]
                    flush(pmm, pphiq)
                    deferred[NQ - 1] = None
                if do_mm:
                    assert n_mm[0] == tot_mm, n_mm[0]

                # out = relu(acc + bias), DMA out
                out_sb = outp.tile([C_OUT, NI], F32, tag="out", name="out_sb")
                if do_mm:
                    nc.scalar.activation(out_sb[:], out_acc[:], AF.Relu,
                                         bias=bias[:, 0:1])
                else:
                    nc.gpsimd.memset(out_sb[:], 0.0)
                nc.sync.dma_start(y_d[:], out_sb[:])

            if repeat == 1:
                body()
            else:
                with tc.For_i(0, repeat, 1,
                              hint_engines=(mybir.EngineType.PE,)) as iv:
                    body(iv)
    nc.compile()
    return nc


def host_prep(features, pos_input, pos_output, extents, kernel, bias):
    """Host-side preprocessing -> per-core input maps."""
    features = np.asarray(features, np.float32)
    pos_input = np.asarray(pos_input, np.float32)
    pos_output = np.asarray(pos_output, np.float32)
    kernel = np.asarray(kernel, np.float32)
    bias = np.asarray(bias, np.float32)
    sc = 2.0 / float(np.asarray(extents).reshape(-1)[0])
    pin = pos_input.astype(np.float64) * sc
    pout = pos_output.astype(np.float64) * sc

    # K2[m0,m1,m2,c,f] in monomial basis; stage-2 lhsT layouts
    K5 = kernel.reshape(3, 3, 3, C_IN, C_OUT)
    K2 = np.einsum("am,bn,co,abcuf->mnouf", M_BASIS, M_BASIS, M_BASIS, K5)
    phi = features @ K2.reshape(K3, C_IN, C_OUT).transpose(1, 0, 2).reshape(
        C_IN, -1)
    phi = phi.reshape(N_IN, K3 * C_OUT).astype(ml_dtypes.bfloat16)

    # r2 hi/lo split (same as v2)
    pin_n2 = np.sum(pin * pin, -1)
    Lr = np.stack([pin_n2, pin[:, 0], pin[:, 1], pin[:, 2],
                   np.ones(N_IN, np.float64)])
    Lh64 = Lr.astype(ml_dtypes.bfloat16).astype(np.float64)
    Ll = (Lr - Lh64).astype(ml_dtypes.bfloat16)
    lhsT15 = np.concatenate(
        [Lr.astype(ml_dtypes.bfloat16), Ll, Lr.astype(ml_dtypes.bfloat16)])

    # pin_sc[p, jt*3+a] = pin[jt*128+p, a]  (f32 per-partition scalars)
    pin_sc = np.ascontiguousarray(
        pin.astype(np.float32).reshape(NJT, 128, 3).transpose(1, 0, 2)
    ).reshape(128, NJT * 3)

    bias_col = bias.reshape(C_OUT, 1).astype(np.float32)

    in_maps = []
    for c in range(N_CORES):
        po = pout[c * NI:(c + 1) * NI]
        po_n2 = np.sum(po * po, -1)
        R = np.stack([np.ones(NI, np.float64), -2.0 * po[:, 0],
                      -2.0 * po[:, 1], -2.0 * po[:, 2], po_n2])
        Rh64 = R.astype(ml_dtypes.bfloat16).astype(np.float64)
        Rl = (R - Rh64).astype(ml_dtypes.bfloat16)
        Rh = R.astype(ml_dtypes.bfloat16)
        rhs15 = np.concatenate([Rh, Rh, Rl])
        nbcast = np.tile(
            np.concatenate([-po[:, 0], -po[:, 1], -po[:, 2]])[None, :],
            (128, 1)).astype(ml_dtypes.bfloat16)
        in_maps.append({
            "pin_sc": pin_sc, "nbcast": nbcast, "phi": phi,
            "lhsT15": lhsT15, "rhs15": rhs15, "bias": bias_col,
        })
    return in_maps


_NC_CACHE = {}


def _get_nc(repeat=1, variant="full"):
    key = (repeat, variant)
    if key not in _NC_CACHE:
        _NC_CACHE[key] = build_nc(repeat, variant)
    return _NC_CACHE[key]


def kernel(features, pos_input, pos_output, extents, kernel, bias):
    nc = _get_nc(1)
    in_maps = host_prep(features, pos_input, pos_output, extents, kernel, bias)
    res = run_bass_kernel_spmd(nc, in_maps, core_ids=list(range(N_CORES)),
                               trace=False)
    out = np.concatenate([res.results[c]["y"] for c in range(N_CORES)], axis=1)
    return np.ascontiguousarray(out.T, dtype=np.float32)
